# revision 45
# baseline (speedup 1.0000x reference)
"""Trainium2 Bass kernel for additive-attention pooling.

reference math:
    scores[b,t] = tanh(q[b]) @ vw_a + tanh(c[b,t]) @ vw_b
    attn        = softmax(where(mask<1, -1e10, scores), axis=t)
    out[b,e]    = sum_t attn[b,t] * c[b,t,e]

Softmax is shift-invariant and the query term is constant over t, so the
output depends only on `context`, `mask` and v_w[E:] (=: w2).  Masked
rows get weight exactly 0, and the mask is ~50% zeros — so the host
compacts each batch to its unmasked rows and casts to bf16, cutting HBM
traffic 4x vs the f32 full-T stream.  The two batches of a core are
packed into ONE padded row stream with a fleet-uniform boundary r0
(batch 0 padded to the max batch-0 count over cores), so a single SPMD
program handles the boundary tile with two zeroed lhsT column sets
(partition-sliced matmuls crash the runtime; full-128-row matmuls with
wrong-batch weights zeroed are equivalent).

Device program, streaming 384-row tiles [128p x (j=3 rows) x 768] bf16:
    th   = tanh(c)                       ACT, one op per tile
    s_j  = sum_e th*w2                   DVE affine_mul_reduce (the wall:
                                         ~0.93us/j; tensor_tensor_reduce
                                         wedges the exec unit, and
                                         mult(2x)+reduce(1x) is slower)
    p    = exp(s)                        ACT, tiny
    num += p^T @ c                       PE bf16 matmuls into f32 PSUM
    p -> HBM                             denominator summed host-side
Pad rows are zero: tanh(0)=0 -> s=0 -> p=1 contribute 0 to num and are
excluded from the host-side denominator, so no mask logic on device.
Tile 0 is processed as three 128-row sub-tiles so the first tanh starts
after a third-size DMA (startup is preamble-dominated).  w2 is loaded
via the ACT HWDGE queue in parallel with tile 0's SP-queue stream.

Sharding: pure data parallel, batch 16 -> 2 per core on 8 cores; w2
replicated (host-broadcast to 128 partitions).  No collectives.
"""

import sys

for _p in ("/opt/trn_rl_repo", "/root/.axon_site/_ro/trn_rl_repo"):
    if _p not in sys.path:
        sys.path.append(_p)

import numpy as np
import ml_dtypes

B, T, E = 16, 4096, 768
NCORES = 8
BPC = B // NCORES  # batches per core
P = 128            # partitions per tile
J = 3              # context rows per partition
RPT = P * J        # rows per tile = 384

_cache = {}


def _build_program(G2, gb, p0):
    """G2 tiles of 384 rows; batch boundary at tile gb, partition p0
    (row r0 = gb*384 + p0*3; p0 == 0 means a clean tile boundary)."""
    import concourse.tile as tile
    from concourse import bacc, mybir

    f32 = mybir.dt.float32
    bf16 = mybir.dt.bfloat16
    AF = mybir.ActivationFunctionType
    S = G2 * J  # score columns
    # warmup: first WT tiles run as 128-row sub-tiles so the first score
    # op starts after a third-size DMA and the DVE pipeline never gaps
    # waiting for the first full-tile tanh; rows must sit inside batch 0
    WT = 1 if gb >= 1 else 0

    nc = bacc.Bacc(
        "TRN2",
        target_bir_lowering=False,
        debug=False,
        enable_asserts=False,
        num_devices=NCORES,
    )
    ctx_d = nc.dram_tensor("ctx", [G2 * RPT, E], bf16, kind="ExternalInput")
    w2_d = nc.dram_tensor("w2rep", [P, E], bf16, kind="ExternalInput")
    num_d = nc.dram_tensor("num", [1, BPC * E], f32, kind="ExternalOutput")
    p_d = nc.dram_tensor("pout", [P, S], bf16, kind="ExternalOutput")

    # first/last tile feeding each PSUM accumulator (start/stop flags)
    first_g = {0: 0, 1: gb}
    last_g = {0: gb if p0 > 0 else gb - 1, 1: G2 - 1}

    with tile.TileContext(nc) as tc:
        with (
            tc.tile_pool(name="const", bufs=1) as const_pool,
            tc.tile_pool(name="cin", bufs=12) as c_pool,
            tc.tile_pool(name="wuc", bufs=3) as wu_pool,
            tc.tile_pool(name="tanh", bufs=3) as t_pool,
            tc.tile_pool(name="wut", bufs=3) as wt_pool,
            tc.tile_pool(name="sb", bufs=2) as sb_pool,
            tc.tile_pool(name="small", bufs=2) as s_pool,
            tc.tile_pool(name="paccum", bufs=2, space="PSUM") as pa_pool,
        ):
            # w2 on the ACT HWDGE queue: lands in parallel with tile 0
            # (on SP it would queue behind context and stall scores)
            w2_rep = const_pool.tile([P, E], bf16)
            nc.scalar.dma_start(w2_rep[:], w2_d[:])

            def load_tile(g):
                c = c_pool.tile([P, J * E], bf16)
                nc.sync.dma_start(
                    c[:].rearrange("p (j e) -> p j e", j=J),
                    ctx_d[g * RPT:(g + 1) * RPT, :].rearrange(
                        "(p j) e -> p j e", j=J
                    ),
                )
                return c

            def load_warm(k):
                c = wu_pool.tile([P, E], bf16)
                nc.sync.dma_start(c[:], ctx_d[k * P:(k + 1) * P, :])
                return c

            sbuf = sb_pool.tile([P, S], f32)
            pbuf = sb_pool.tile([P, S], bf16)
            acc0 = pa_pool.tile([1, E], f32)
            acc1 = pa_pool.tile([1, E], f32)
            accs = [acc0, acc1]

            # boundary tile: partition-sliced matmuls wedge the device,
            # so batch-split via two zeroed lhsT column sets instead
            # (full-128-partition matmuls; wrong-batch rows weigh 0)
            if p0 > 0:
                pz = const_pool.tile([P, 2 * J], bf16)
                nc.gpsimd.memset(pz[:], 0.0)

            def mm_pair(acc, lhsT, c, e0, st, sp):
                nc.tensor.matmul(
                    acc[:, 0:512], lhsT=lhsT,
                    rhs=c[:, e0:e0 + 512], start=st, stop=sp,
                )
                nc.tensor.matmul(
                    acc[:, 512:E], lhsT=lhsT,
                    rhs=c[:, e0 + 512:e0 + E], start=st, stop=sp,
                )

            def score_exp(th, sl, col):
                nc.vector.affine_mul_reduce(
                    th[:, sl], sbuf[:, col:col + 1],
                    th[:, sl], w2_rep[:], 1.0, 0.0,
                )

            def process(g, c, th):
                last = g == G2 - 1
                split = g == gb and p0 > 0
                jgrp = [[j] for j in range(J)] if last else [list(range(J))]
                for grp in jgrp:
                    for j in grp:
                        score_exp(th, slice(j * E, (j + 1) * E), g * J + j)
                    c0, c1 = g * J + grp[0], g * J + grp[-1] + 1
                    nc.scalar.activation(
                        pbuf[:, c0:c1], sbuf[:, c0:c1], AF.Exp
                    )
                    if split:
                        m0, m1 = grp[0], grp[-1] + 1
                        nc.gpsimd.tensor_copy(
                            pz[0:p0, m0:m1], pbuf[0:p0, c0:c1]
                        )
                        nc.gpsimd.tensor_copy(
                            pz[p0:P, J + m0:J + m1], pbuf[p0:P, c0:c1]
                        )
                    for j in grp:
                        if split:
                            st0 = g == first_g[0] and j == 0
                            st1 = g == first_g[1] and j == 0
                            sp0 = g == last_g[0] and j == J - 1
                            mm_pair(acc0, pz[:, j:j + 1], c, j * E,
                                    st0, sp0)
                            mm_pair(acc1, pz[:, J + j:J + j + 1], c, j * E,
                                    st1, False)
                        else:
                            ai = 0 if g < gb else 1
                            st = g == first_g[ai] and j == 0 and grp[0] == 0
                            sp = g == last_g[ai] and j == J - 1
                            mm_pair(accs[ai], pbuf[:, c0 + j - grp[0]:
                                                   c0 + j - grp[0] + 1],
                                    c, j * E, st, sp)

            def process_warm(k, c, th):
                score_exp(th, slice(0, E), k)
                nc.scalar.activation(
                    pbuf[:, k:k + 1], sbuf[:, k:k + 1], AF.Exp
                )
                mm_pair(acc0, pbuf[:, k:k + 1], c, 0, k == 0, False)

            # software pipeline: tanh(k+1) emitted before the score
            # chain of item k, so ACT never queues an exp that waits on
            # DVE ahead of ready tanh work
            items = []  # (kind, key, c, th)
            for k in range(3 * WT):
                items.append(("w", k, load_warm(k)))
            for g in range(WT, min(WT + 4, G2)):
                items.append(("t", g, load_tile(g)))
            rest = range(min(WT + 4, G2), G2)

            pend = None

            def step(kind, key, c):
                nonlocal pend
                if kind == "w":
                    th = wt_pool.tile([P, E], bf16)
                    nc.scalar.activation(th[:], c[:], AF.Tanh)
                else:
                    th = t_pool.tile([P, J * E], bf16)
                    nc.scalar.activation(th[:], c[:], AF.Tanh)
                if pend is not None:
                    pk, pkey, pc, pth = pend
                    if pk == "w":
                        process_warm(pkey, pc, pth)
                    else:
                        process(pkey, pc, pth)
                pend = (kind, key, c, th)

            for kind, key, c in items:
                step(kind, key, c)
            for g in rest:
                step("t", g, load_tile(g))
            pk, pkey, pc, pth = pend
            if pk == "w":
                process_warm(pkey, pc, pth)
            else:
                process(pkey, pc, pth)

            # denominator values + numerators out. acc0's drain rides the
            # ACT engine (idle after the final tanh) so only acc1's
            # 0.95us PSUM copy sits in the serial tail on DVE
            nc.sync.dma_start(p_d[:], pbuf[:])
            osb = s_pool.tile([1, BPC * E], f32)
            nc.scalar.copy(osb[:, 0:E], acc0[:])
            nc.vector.tensor_copy(osb[:, E:BPC * E], acc1[:])
            nc.sync.dma_start(num_d[:], osb[:])

    nc.compile()
    return nc


def _get_program(G2, gb, p0):
    key = ("nc", G2, gb, p0)
    if key not in _cache:
        _cache[key] = _build_program(G2, gb, p0)
    return _cache[key]


def _prepare(context, mask, v_w):
    """Compact unmasked rows, pack each core's two batches into one
    padded stream with a fleet-uniform boundary, cast to bf16."""
    bf16 = ml_dtypes.bfloat16
    m = np.asarray(mask)
    counts = m.sum(axis=1).astype(np.int64)
    n0 = counts[0::2]  # first batch of each core
    n1 = counts[1::2]
    # uniform boundary r0: PE matmuls split partitions at p0 = (r0%384)/3,
    # and a [p0:128] lhsT needs base partition 0, 64 or 96 -> r0 mod 384
    # must be 0, 192 or 288
    nmax = int(n0.max())
    r0 = min(
        x
        for k in range(nmax // RPT, nmax // RPT + 2)
        for off in (0, 192, 288)
        if (x := k * RPT + off) >= nmax
    )
    T2 = int(max(RPT, -(-(r0 + n1.max()) // RPT) * RPT))
    G2 = T2 // RPT
    gb, p0 = r0 // RPT, (r0 % RPT) // 3

    ctx = np.asarray(context, dtype=np.float32)
    in_maps = []
    w2 = np.asarray(v_w[E:], dtype=np.float32).astype(bf16)
    w2_rep = np.ascontiguousarray(np.broadcast_to(w2, (P, E)))
    for i in range(NCORES):
        ctx2 = np.zeros((T2, E), dtype=bf16)
        ia = np.flatnonzero(m[2 * i])
        ib = np.flatnonzero(m[2 * i + 1])
        ctx2[:len(ia)] = ctx[2 * i, ia].astype(bf16)
        ctx2[r0:r0 + len(ib)] = ctx[2 * i + 1, ib].astype(bf16)
        in_maps.append({"ctx": ctx2, "w2rep": w2_rep})
    return (G2, gb, p0), in_maps, (counts, r0, T2)


def _finish(res, params, meta):
    """Gather per-core outputs, host-side softmax denominator + divide."""
    G2, gb, p0 = params
    counts, r0, T2 = meta
    WT = 1 if gb >= 1 else 0
    S = G2 * J

    # column/partition -> packed row index map
    tmap = np.empty(T2, dtype=np.int64)
    pidx = np.arange(P)
    for col in range(S):
        g, j = col // J, col % J
        if col < 3 * WT:
            t = col * P + pidx          # warmup sub-tiles: partition-major
        else:
            t = g * RPT + pidx * J + j
        tmap[t] = col * P + pidx  # flat index into pout^T

    out = np.empty((B, E), dtype=np.float32)
    for i in range(NCORES):
        num = np.asarray(res.results[i]["num"]).astype(np.float32).reshape(BPC, E)
        pout = np.asarray(res.results[i]["pout"]).astype(np.float32)
        pv = pout.T.reshape(-1)[tmap]  # p value per packed row t
        na, nb = counts[2 * i], counts[2 * i + 1]
        den0 = pv[:na].sum(dtype=np.float64)
        den1 = pv[r0:r0 + nb].sum(dtype=np.float64)
        out[2 * i] = num[0] / np.float32(den0)
        out[2 * i + 1] = num[1] / np.float32(den1)
    return out


def kernel(query, context, mask, v_w):
    import time
    from concourse.bass_utils import run_bass_kernel_spmd

    params, in_maps, meta = _prepare(context, mask, v_w)
    nc = _get_program(*params)
    last_err = None
    for attempt in range(3):
        try:
            res = run_bass_kernel_spmd(nc, in_maps, list(range(NCORES)))
            return _finish(res, params, meta)
        except Exception as e:  # transient axon/device hiccups
            last_err = e
            time.sleep(5)
    raise last_err
